# revision 17
# baseline (speedup 1.0000x reference)
"""BGNN context message-passing kernel for 8 TRN2 NeuronCores.

Sharding: edges (rel dim) across 8 cores, nodes sharded for the update/
collective phase.  Per iteration: per-core scatter-add accumulators are
ReduceScattered (summed, node-sharded), each core updates its node shard,
and the new node table is AllGathered for the next iteration's gathers.

Hidden states are kept edge-major ([128 edges, H] tiles) in SBUF; matmul
operands are produced feature-major on the fly with PE transposes (relu
fused into the PSUM->SBUF eviction).  All matmuls run in bf16 with fp32
PSUM accumulation.  The scatter-add runs on the DMA engines via indirect
DMA with compute_op=add into an interleaved [2N, ACW] accumulator whose
column 1024 carries the segment counts.
"""

import numpy as np
import ml_dtypes

import concourse.bass as bass
import concourse.mybir as mybir
import concourse.tile as tile
from concourse import bacc
from concourse.bass_utils import run_bass_kernel_spmd
from concourse.masks import make_identity

NCORES = 8
N = 4096
E = 32768
PDIM = 4096
H = 1024
F = 64
NITER = 2
EPS = 1e-5

EC = E // NCORES          # 4096 edges per core
NSH = N // NCORES         # 512 nodes per core
ET = EC // 128            # 32 edge tiles
NT = NSH // 128           # 4 node tiles
KP = PDIM // 128          # 32 contraction chunks for down-proj
KH = H // 128             # 8 contraction chunks for H
ACW = 1040                # acc row: 1024 msg cols + count col @1024 + pad

BF = mybir.dt.bfloat16
F32 = mybir.dt.float32
I32 = mybir.dt.int32
RG = [list(range(NCORES))]
BF_NP = ml_dtypes.bfloat16


def _build(nc):
    def din(name, shape, dtype):
        return nc.dram_tensor(name, shape, dtype, kind="ExternalInput")

    relft = din("rel_feat_t", [PDIM, EC], BF)       # rel_feat shard, transposed
    objft = din("obj_feat_t", [PDIM, NSH], BF)      # obj_feat shard, transposed
    wrd = din("w_rel_down", [PDIM, H], BF)
    wod = din("w_obj_down", [PDIM, H], BF)
    brd = din("b_rel_down_rep", [128, H], BF)       # bias replicated over partitions
    bod = din("b_obj_down_rep", [128, H], BF)
    sidx = din("sub_idx", [EC, 1], I32)
    oidx = din("obj_idx", [EC, 1], I32)
    sidx2 = din("sub_idx2", [EC, 1], I32)           # 2*sub_idx (acc rows)
    oidx2 = din("obj_idx2", [EC, 1], I32)           # 2*obj_idx+1
    w_pair1 = din("w_pair1", [2 * H, 2 * F], BF)    # [w_s2p | w_p2s_reordered]
    w_pair2 = din("w_pair2", [2 * H, 2 * F], BF)    # [w_o2p | w_p2o_reordered]
    bz1 = din("bz_pair1", [2 * F, 1], F32)          # sigmoid bias (concat)
    bz2 = din("bz_pair2", [2 * F, 1], F32)
    ones_blk = din("ones_blk", [2 * F, 2], F32)     # block 1/128 for gate means
    wih_rel = din("wih_relf", [H, H], BF)
    whh_rel = din("whh_relf", [H, H], BF)
    wih_obj = din("wih_objf", [H, H], BF)
    whh_obj = din("whh_objf", [H, H], BF)
    bf_rel = din("bfus_rel_rep", [128, H], F32)     # bih+bhh replicated
    bf_obj = din("bfus_obj_rep", [128, H], F32)

    out_obj = nc.dram_tensor("out_obj", [NSH, H], F32, kind="ExternalOutput")
    out_rel = nc.dram_tensor("out_rel", [EC, H], F32, kind="ExternalOutput")

    with tile.TileContext(nc) as tc:
        with (
            tc.tile_pool(name="const", bufs=1) as const,
            tc.tile_pool(name="relbuf", bufs=ET) as relbuf,
            tc.tile_pool(name="ownbuf", bufs=2) as ownbuf,
            tc.tile_pool(name="stream", bufs=3) as stream,
            tc.tile_pool(name="wo", bufs=4) as wo,
            tc.tile_pool(name="eb", bufs=8) as eb,
            tc.tile_pool(name="ft", bufs=8) as ft,
            tc.tile_pool(name="fw", bufs=4) as fw,
            tc.tile_pool(name="small", bufs=4) as small,
            tc.tile_pool(name="dram", bufs=2, space="DRAM") as dram,
        ):
            ident = const.tile([128, 128], BF)
            make_identity(nc, ident)

            # resident weights / constants
            w1_sb = const.tile([128, 2 * KH, 2 * F], BF)
            nc.sync.dma_start(w1_sb, w_pair1.rearrange("(o p) m -> p o m", p=128))
            w2_sb = const.tile([128, 2 * KH, 2 * F], BF)
            nc.sync.dma_start(w2_sb, w_pair2.rearrange("(o p) m -> p o m", p=128))
            bz1_sb = const.tile([128, 1], F32)
            nc.sync.dma_start(bz1_sb, bz1[:])
            bz2_sb = const.tile([128, 1], F32)
            nc.sync.dma_start(bz2_sb, bz2[:])
            onesb_sb = const.tile([128, 2], F32)
            nc.sync.dma_start(onesb_sb, ones_blk[:])
            brd_sb = const.tile([128, H], BF)
            nc.sync.dma_start(brd_sb, brd[:])
            bod_sb = const.tile([128, H], BF)
            nc.sync.dma_start(bod_sb, bod[:])
            bfr_sb = const.tile([128, H], F32)
            nc.sync.dma_start(bfr_sb, bf_rel[:])
            bfo_sb = const.tile([128, H], F32)
            nc.sync.dma_start(bfo_sb, bf_obj[:])
            wih_r_sb = const.tile([128, KH, H], BF)
            nc.sync.dma_start(wih_r_sb, wih_rel.rearrange("(o p) m -> p o m", p=128))
            whh_r_sb = const.tile([128, KH, H], BF)
            nc.sync.dma_start(whh_r_sb, whh_rel.rearrange("(o p) m -> p o m", p=128))
            sidx_sb = const.tile([128, ET], I32)
            nc.sync.dma_start(sidx_sb, sidx.rearrange("(o p) x -> p (o x)", p=128))
            oidx_sb = const.tile([128, ET], I32)
            nc.sync.dma_start(oidx_sb, oidx.rearrange("(o p) x -> p (o x)", p=128))
            sidx2_sb = const.tile([128, ET], I32)
            nc.sync.dma_start(sidx2_sb,
                              sidx2.rearrange("(o p) x -> p (o x)", p=128))
            oidx2_sb = const.tile([128, ET], I32)
            nc.sync.dma_start(oidx2_sb,
                              oidx2.rearrange("(o p) x -> p (o x)", p=128))
            zrow = const.tile([128, ACW], BF)
            nc.vector.memset(zrow, 0.0)

            # ---------------- down projections ----------------
            rel_tiles = [relbuf.tile([128, H], BF, tag="relt", name=f"relt{i}")
                         for i in range(ET)]

            def down_proj(psD, feat_t, wdown, bias_rep, ntiles, out_tiles):
                # groups of 4 output tiles; stream weight k-chunks
                for g in range(0, ntiles, 4):
                    gw = min(4, ntiles - g)
                    pts = [psD.tile([128, H], F32, tag="dp", name=f"dp{i}")
                           for i in range(gw)]
                    for k in range(KP):
                        wt = stream.tile([128, H], BF, tag="wdown")
                        nc.sync.dma_start(wt, wdown[k * 128:(k + 1) * 128, :])
                        xt = stream.tile([128, 128 * gw], BF, tag="xdown")
                        nc.sync.dma_start(
                            xt, feat_t[k * 128:(k + 1) * 128,
                                       g * 128:g * 128 + 128 * gw])
                        for i in range(gw):
                            for hh in range(2):
                                nc.tensor.matmul(
                                    out=pts[i][:, hh * 512:(hh + 1) * 512],
                                    lhsT=xt[:, i * 128:(i + 1) * 128],
                                    rhs=wt[:, hh * 512:(hh + 1) * 512],
                                    start=(k == 0), stop=(k == KP - 1))
                    for i in range(gw):
                        ot = out_tiles[g + i]
                        nc.vector.tensor_tensor(
                            out=ot, in0=pts[i], in1=bias_rep,
                            op=mybir.AluOpType.add)
                        nc.vector.tensor_scalar_max(ot, ot, 0.0)

            obj_nm = [eb.tile([128, H], BF, tag="eb", name=f"objnm{i}")
                      for i in range(NT)]
            with tc.tile_pool(name="psD", bufs=4, space="PSUM") as psD:
                down_proj(psD, relft, wrd, brd_sb, ET, rel_tiles)
                down_proj(psD, objft, wod, bod_sb, NT, obj_nm)

            from contextlib import ExitStack
            _ps_stack = ExitStack()
            psA = _ps_stack.enter_context(
                tc.tile_pool(name="psA", bufs=4, space="PSUM"))
            psT = _ps_stack.enter_context(
                tc.tile_pool(name="psT", bufs=4, space="PSUM"))

            # own_t: relu'd hidden, feature-major [128, KH, NSH]
            own_t = ownbuf.tile([128, KH, NSH], BF, tag="own")
            ag_in = dram.tile([NSH, H], BF)
            for ntl in range(NT):
                nc.sync.dma_start(ag_in[ntl * 128:(ntl + 1) * 128, :], obj_nm[ntl])
                for c in range(KH):
                    tp = psT.tile([128, 128], BF, tag="tp")
                    nc.tensor.transpose(tp, obj_nm[ntl][:, c * 128:(c + 1) * 128],
                                        ident)
                    nc.scalar.activation(
                        own_t[:, c, ntl * 128:(ntl + 1) * 128], tp,
                        mybir.ActivationFunctionType.Relu)

            table = dram.tile([N, H], BF, addr_space="Shared")
            nc.gpsimd.collective_compute(
                "AllGather", mybir.AluOpType.bypass, replica_groups=RG,
                ins=[ag_in.opt()], outs=[table.opt()])

            # ---------------- iterations ----------------
            for it in range(NITER):
                last = it == NITER - 1
                acc = dram.tile([2 * N, ACW], BF, tag="acc")
                for r in range(2 * N // 128):
                    nc.sync.dma_start(acc[r * 128:(r + 1) * 128, :], zrow)

                for et in range(ET):
                    relt = rel_tiles[et]
                    subh = eb.tile([128, H], BF, tag="eb", name="subh")
                    nc.gpsimd.indirect_dma_start(
                        out=subh, out_offset=None, in_=table[:, :],
                        in_offset=bass.IndirectOffsetOnAxis(
                            ap=sidx_sb[:, et:et + 1], axis=0))
                    objh = eb.tile([128, H], BF, tag="eb", name="objh")
                    nc.gpsimd.indirect_dma_start(
                        out=objh, out_offset=None, in_=table[:, :],
                        in_offset=bass.IndirectOffsetOnAxis(
                            ap=oidx_sb[:, et:et + 1], axis=0))

                    # layernorm stats + gates for the two MPU pairs
                    gts = []
                    for pair_i, (oth, w_sb, bz_sb) in enumerate(
                            [(subh, w1_sb, bz1_sb), (objh, w2_sb, bz2_sb)]):
                        st = small.tile([128, 4, 6], F32, tag="bnst")
                        r3 = relt.rearrange("p (a b) -> p a b", b=512)
                        o3 = oth.rearrange("p (a b) -> p a b", b=512)
                        nc.vector.bn_stats(st[:, 0, :], r3[:, 0, :])
                        nc.vector.bn_stats(st[:, 1, :], r3[:, 1, :])
                        nc.vector.bn_stats(st[:, 2, :], o3[:, 0, :])
                        nc.vector.bn_stats(st[:, 3, :], o3[:, 1, :])
                        mv = small.tile([128, 2], F32, tag="bnmv")
                        nc.vector.bn_aggr(mv, st)
                        sd = small.tile([128, 1], F32, tag="sd")
                        nc.vector.tensor_scalar_add(sd, mv[:, 1:2], EPS)
                        nc.scalar.sqrt(sd, sd)
                        rstd = small.tile([128, 1], F32, tag="rstd")
                        nc.vector.reciprocal(rstd, sd)

                        # normalized tiles (edge-major)
                        xa = eb.tile([128, H], BF, tag="eb", name="xa")
                        nc.vector.tensor_scalar(
                            out=xa, in0=relt, scalar1=mv[:, 0:1], scalar2=rstd,
                            op0=mybir.AluOpType.subtract, op1=mybir.AluOpType.mult)
                        xb = eb.tile([128, H], BF, tag="eb", name="xb")
                        nc.vector.tensor_scalar(
                            out=xb, in0=oth, scalar1=mv[:, 0:1], scalar2=rstd,
                            op0=mybir.AluOpType.subtract, op1=mybir.AluOpType.mult)

                        # gate logits: transpose+relu chunk, then matmul
                        zp = psT.tile([128, 128], F32, tag="tp")
                        for c in range(2 * KH):
                            src = xa if c < KH else xb
                            cc = c if c < KH else c - KH
                            tp = psT.tile([128, 128], BF, tag="tp")
                            nc.tensor.transpose(
                                tp, src[:, cc * 128:(cc + 1) * 128], ident)
                            xch = ft.tile([128, 128], BF, tag="ft", name="xch")
                            nc.scalar.activation(
                                xch, tp, mybir.ActivationFunctionType.Relu)
                            nc.tensor.matmul(
                                out=zp, lhsT=w_sb[:, c, :], rhs=xch,
                                start=(c == 0), stop=(c == 2 * KH - 1))
                        zs = small.tile([128, 128], F32, tag="zs")
                        nc.scalar.activation(
                            zs, zp, mybir.ActivationFunctionType.Sigmoid,
                            bias=bz_sb)
                        gt_ps = psT.tile([128, 2], F32, tag="tp")
                        nc.tensor.matmul(out=gt_ps, lhsT=zs, rhs=onesb_sb,
                                         start=True, stop=True)
                        gt = small.tile([128, 2], F32, tag="gt")
                        nc.vector.tensor_copy(gt, gt_ps)
                        gts.append(gt)

                    # scatter messages (gates pre-scaled by 1/128 = mean*0.5)
                    for gt, idx_sb in ((gts[0], sidx2_sb),
                                       (gts[1], oidx2_sb)):
                        m = eb.tile([128, ACW], BF, tag="eb", name="mscat")
                        nc.vector.tensor_scalar_mul(m[:, 0:H], relt, gt[:, 1:2])
                        nc.vector.memset(m[:, H:H + 1], 1.0)
                        nc.vector.memset(m[:, H + 1:ACW], 0.0)
                        nc.gpsimd.indirect_dma_start(
                            out=acc[:, :], out_offset=bass.IndirectOffsetOnAxis(
                                ap=idx_sb[:, et:et + 1], axis=0),
                            in_=m, in_offset=None,
                            compute_op=mybir.AluOpType.add)

                    # rel message: subh*g_s2p/2 + objh*g_o2p/2
                    t1 = eb.tile([128, H], BF, tag="eb", name="t1")
                    nc.vector.tensor_scalar_mul(t1, subh, gts[0][:, 0:1])
                    msg = eb.tile([128, H], BF, tag="eb", name="msg")
                    nc.vector.tensor_scalar_mul(msg, objh, gts[1][:, 0:1])
                    nc.vector.tensor_tensor(out=msg, in0=msg, in1=t1,
                                            op=mybir.AluOpType.add)

                    # rel fusion: per chunk transpose+relu msg / relt, matmul
                    fph = [psA.tile([128, 512], F32, tag="fus", name=f"fph{i}")
                           for i in range(2)]
                    for c in range(KH):
                        tp = psT.tile([128, 128], BF, tag="tp")
                        nc.tensor.transpose(tp, msg[:, c * 128:(c + 1) * 128],
                                            ident)
                        mch = ft.tile([128, 128], BF, tag="ft", name="mch")
                        nc.scalar.activation(
                            mch, tp, mybir.ActivationFunctionType.Relu)
                        tp2 = psT.tile([128, 128], BF, tag="tp")
                        nc.tensor.transpose(tp2, relt[:, c * 128:(c + 1) * 128],
                                            ident)
                        hch = ft.tile([128, 128], BF, tag="ft", name="hch")
                        nc.scalar.activation(
                            hch, tp2, mybir.ActivationFunctionType.Relu)
                        for hh in range(2):
                            sl = slice(hh * 512, (hh + 1) * 512)
                            nc.tensor.matmul(
                                out=fph[hh], lhsT=mch,
                                rhs=wih_r_sb[:, c, sl],
                                start=(c == 0), stop=False)
                            nc.tensor.matmul(
                                out=fph[hh], lhsT=hch,
                                rhs=whh_r_sb[:, c, sl],
                                start=False, stop=(c == KH - 1))
                    if last:
                        fo = eb.tile([128, H], F32, tag="f32w", name="fo")
                        for hh in range(2):
                            sl = slice(hh * 512, (hh + 1) * 512)
                            nc.vector.tensor_tensor(
                                out=fo[:, sl], in0=fph[hh], in1=bfr_sb[:, sl],
                                op=mybir.AluOpType.add)
                        nc.sync.dma_start(out_rel[et * 128:(et + 1) * 128, :], fo)
                    else:
                        for hh in range(2):
                            sl = slice(hh * 512, (hh + 1) * 512)
                            nc.vector.tensor_tensor(
                                out=relt[:, sl], in0=fph[hh], in1=bfr_sb[:, sl],
                                op=mybir.AluOpType.add)

                # ---- node update ----
                rs_a = dram.tile([2 * NSH, ACW], BF, tag="rsa")
                nc.gpsimd.collective_compute(
                    "ReduceScatter", mybir.AluOpType.add, replica_groups=RG,
                    ins=[acc.opt()], outs=[rs_a.opt()])
                rs_3 = rs_a.rearrange("(n t) w -> n t w", t=2)

                if not last:
                    ag_in2 = dram.tile([NSH, H], BF, tag="agin")
                    new_own = ownbuf.tile([128, KH, NSH], BF, tag="own")
                for ntl in range(NT):
                    asb = eb.tile([128, ACW], BF, tag="eb", name="asb")
                    nc.sync.dma_start(asb, rs_3[ntl * 128:(ntl + 1) * 128, 0, :])
                    aob = eb.tile([128, ACW], BF, tag="eb", name="aob")
                    nc.sync.dma_start(aob, rs_3[ntl * 128:(ntl + 1) * 128, 1, :])
                    msgn = eb.tile([128, H], F32, tag="f32w", name="msgn")
                    tmpn = eb.tile([128, H], F32, tag="f32w", name="tmpn")
                    for src, dst in ((asb, msgn), (aob, tmpn)):
                        cnt = small.tile([128, 1], F32, tag="cnt")
                        nc.vector.tensor_copy(cnt, src[:, H:H + 1])
                        nc.vector.tensor_scalar_max(cnt, cnt, 1.0)
                        rc = small.tile([128, 1], F32, tag="rc")
                        nc.vector.reciprocal(rc, cnt)
                        nc.vector.tensor_scalar_mul(dst, src[:, 0:H], rc)
                    nc.vector.tensor_tensor(out=msgn, in0=msgn, in1=tmpn,
                                            op=mybir.AluOpType.add)
                    msgr = eb.tile([128, H], BF, tag="eb", name="msgr")
                    nc.vector.tensor_scalar_max(msgr, msgn, 0.0)
                    fph = [psA.tile([128, 512], F32, tag="fus", name=f"fphn{i}")
                           for i in range(2)]
                    for c in range(KH):
                        tp = psT.tile([128, 128], BF, tag="tp")
                        nc.tensor.transpose(tp, msgr[:, c * 128:(c + 1) * 128],
                                            ident)
                        mch = ft.tile([128, 128], BF, tag="ft", name="mchn")
                        nc.scalar.activation(
                            mch, tp, mybir.ActivationFunctionType.Copy)
                        for hh in range(2):
                            sl = slice(hh * 512, (hh + 1) * 512)
                            wi = wo.tile([128, 512], BF, tag="wo", name="wi")
                            nc.sync.dma_start(
                                wi, wih_obj[c * 128:(c + 1) * 128, sl])
                            wh = wo.tile([128, 512], BF, tag="wo", name="wh")
                            nc.sync.dma_start(
                                wh, whh_obj[c * 128:(c + 1) * 128, sl])
                            nc.tensor.matmul(
                                out=fph[hh], lhsT=mch, rhs=wi,
                                start=(c == 0), stop=False)
                            nc.tensor.matmul(
                                out=fph[hh],
                                lhsT=own_t[:, c, ntl * 128:(ntl + 1) * 128],
                                rhs=wh,
                                start=False, stop=(c == KH - 1))
                    if last:
                        onew = eb.tile([128, H], F32, tag="f32w", name="onew")
                        for hh in range(2):
                            sl = slice(hh * 512, (hh + 1) * 512)
                            nc.vector.tensor_tensor(
                                out=onew[:, sl], in0=fph[hh], in1=bfo_sb[:, sl],
                                op=mybir.AluOpType.add)
                        nc.sync.dma_start(out_obj[ntl * 128:(ntl + 1) * 128, :],
                                          onew)
                    else:
                        onb = eb.tile([128, H], BF, tag="eb", name="onb")
                        for hh in range(2):
                            sl = slice(hh * 512, (hh + 1) * 512)
                            nc.vector.tensor_tensor(
                                out=onb[:, sl], in0=fph[hh], in1=bfo_sb[:, sl],
                                op=mybir.AluOpType.add)
                        nc.sync.dma_start(
                            ag_in2[ntl * 128:(ntl + 1) * 128, :], onb)
                        for c in range(KH):
                            tp = psT.tile([128, 128], BF, tag="tp")
                            nc.tensor.transpose(
                                tp, onb[:, c * 128:(c + 1) * 128], ident)
                            nc.scalar.activation(
                                new_own[:, c, ntl * 128:(ntl + 1) * 128], tp,
                                mybir.ActivationFunctionType.Relu)
                if not last:
                    table2 = dram.tile([N, H], BF, tag="table2", addr_space="Shared")
                    nc.gpsimd.collective_compute(
                        "AllGather", mybir.AluOpType.bypass, replica_groups=RG,
                        ins=[ag_in2.opt()], outs=[table2.opt()])
                    table = table2
                    own_t = new_own
            _ps_stack.close()
    return nc


def _prep_inputs(inputs):
    f = {k: np.asarray(v) for k, v in inputs.items()}
    relT = np.ascontiguousarray(f["rel_feat"].astype(BF_NP).T)      # [PDIM, E]
    objT = np.ascontiguousarray(f["obj_feat"].astype(BF_NP).T)      # [PDIM, N]

    def reord(w):
        return np.concatenate([w[H:], w[:H]], axis=0)

    w1 = np.concatenate([f["w_s2p"], reord(f["w_p2s"])], axis=1).astype(BF_NP)
    w2 = np.concatenate([f["w_o2p"], reord(f["w_p2o"])], axis=1).astype(BF_NP)
    bz1 = np.concatenate([f["b_s2p"], f["b_p2s"]]).astype(np.float32)[:, None]
    bz2 = np.concatenate([f["b_o2p"], f["b_p2o"]]).astype(np.float32)[:, None]
    ones_blk = np.zeros((2 * F, 2), np.float32)
    ones_blk[:F, 0] = 1.0 / 128.0
    ones_blk[F:, 1] = 1.0 / 128.0

    def rep(b, dt=np.float32):
        return np.tile(np.asarray(b).astype(dt)[None, :], (128, 1))

    common = {
        "w_rel_down": f["w_rel_down"].astype(BF_NP),
        "w_obj_down": f["w_obj_down"].astype(BF_NP),
        "b_rel_down_rep": rep(f["b_rel_down"], BF_NP),
        "b_obj_down_rep": rep(f["b_obj_down"], BF_NP),
        "w_pair1": w1, "w_pair2": w2,
        "bz_pair1": bz1, "bz_pair2": bz2, "ones_blk": ones_blk,
        "wih_relf": f["wih_relf"].astype(BF_NP),
        "whh_relf": f["whh_relf"].astype(BF_NP),
        "wih_objf": f["wih_objf"].astype(BF_NP),
        "whh_objf": f["whh_objf"].astype(BF_NP),
        "bfus_rel_rep": rep(f["bih_relf"] + f["bhh_relf"]),
        "bfus_obj_rep": rep(f["bih_objf"] + f["bhh_objf"]),
    }
    maps = []
    for c in range(NCORES):
        m = dict(common)
        m["rel_feat_t"] = np.ascontiguousarray(relT[:, c * EC:(c + 1) * EC])
        m["obj_feat_t"] = np.ascontiguousarray(objT[:, c * NSH:(c + 1) * NSH])
        si = f["sub_idx"][c * EC:(c + 1) * EC].astype(np.int32)[:, None]
        oi = f["obj_idx"][c * EC:(c + 1) * EC].astype(np.int32)[:, None]
        m["sub_idx"] = np.ascontiguousarray(si)
        m["obj_idx"] = np.ascontiguousarray(oi)
        m["sub_idx2"] = np.ascontiguousarray(2 * si)
        m["obj_idx2"] = np.ascontiguousarray(2 * oi + 1)
        maps.append(m)
    return maps


def _run(inputs, trace=False):
    nc = bacc.Bacc(None, target_bir_lowering=False)
    _build(nc)
    nc.compile()
    maps = _prep_inputs(inputs)
    res = run_bass_kernel_spmd(nc, maps, core_ids=list(range(NCORES)),
                               trace=trace)
    outs = res.results
    obj = np.concatenate([np.asarray(outs[c]["out_obj"], np.float32)
                          for c in range(NCORES)], axis=0)
    rel = np.concatenate([np.asarray(outs[c]["out_rel"], np.float32)
                          for c in range(NCORES)], axis=0)
    full = np.concatenate([obj, rel], axis=0)
    return full, res


def kernel(**inputs):
    full, _ = _run(inputs, trace=False)
    return full


# revision 22
# speedup vs baseline: 14.7364x; 14.7364x over previous
"""BGNN context message-passing kernel for 8 TRN2 NeuronCores.

Sharding: edges (rel dim) across 8 cores, nodes sharded for the update/
collective phase.  Per iteration: per-core scatter-add accumulators are
ReduceScattered (summed, node-sharded), each core updates its node shard,
and the new node table is AllGathered for the next iteration's gathers.

Hidden states are kept edge-major ([128 edges, H] tiles) in SBUF; matmul
operands are produced feature-major on the fly with PE transposes (relu
fused into the PSUM->SBUF eviction).  All matmuls run in bf16 with fp32
PSUM accumulation.  The scatter-add runs on the DMA engines via indirect
DMA with compute_op=add into an interleaved [2N, ACW] accumulator whose
column 1024 carries the segment counts.
"""

import numpy as np
import ml_dtypes

import concourse.bass as bass
import concourse.mybir as mybir
import concourse.tile as tile
from concourse import bacc
from concourse.bass_utils import run_bass_kernel_spmd
from concourse.masks import make_identity

NCORES = 8
N = 4096
E = 32768
PDIM = 4096
H = 1024
F = 64
NITER = 2
EPS = 1e-5

EC = E // NCORES          # 4096 edges per core
NSH = N // NCORES         # 512 nodes per core
ET = EC // 128            # 32 edge tiles
NT = NSH // 128           # 4 node tiles
KP = PDIM // 128          # 32 contraction chunks for down-proj
KH = H // 128             # 8 contraction chunks for H
ACW = 1040                # acc row: 1024 msg cols + count col @1024 + pad

BF = mybir.dt.bfloat16
F32 = mybir.dt.float32
I32 = mybir.dt.int32
RG = [list(range(NCORES))]
BF_NP = ml_dtypes.bfloat16


def _build(nc):
    def din(name, shape, dtype):
        return nc.dram_tensor(name, shape, dtype, kind="ExternalInput")

    relft = din("rel_feat_t", [PDIM, EC], BF)       # rel_feat shard, transposed
    objft = din("obj_feat_t", [PDIM, NSH], BF)      # obj_feat shard, transposed
    wrd = din("w_rel_down", [PDIM, H], BF)
    wod = din("w_obj_down", [PDIM, H], BF)
    brd = din("b_rel_down_rep", [128, H], BF)       # bias replicated over partitions
    bod = din("b_obj_down_rep", [128, H], BF)
    sidx = din("sub_idx", [EC, 1], I32)
    oidx = din("obj_idx", [EC, 1], I32)
    sidx2 = din("sub_idx2", [EC, 1], I32)           # 2*sub_idx (acc rows)
    oidx2 = din("obj_idx2", [EC, 1], I32)           # 2*obj_idx+1
    w_pair1 = din("w_pair1", [2 * H, 2 * F], BF)    # [w_s2p | w_p2s_reordered]
    w_pair2 = din("w_pair2", [2 * H, 2 * F], BF)    # [w_o2p | w_p2o_reordered]
    bz1 = din("bz_pair1", [2 * F, 1], F32)          # sigmoid bias (concat)
    bz2 = din("bz_pair2", [2 * F, 1], F32)
    ones_blk = din("ones_blk", [2 * F, 2], F32)     # block 1/128 for gate means
    wih_rel = din("wih_relf", [H, H], BF)
    whh_rel = din("whh_relf", [H, H], BF)
    wih_obj = din("wih_objf", [H, H], BF)
    whh_obj = din("whh_objf", [H, H], BF)
    bf_rel = din("bfus_rel_rep", [128, H], F32)     # bih+bhh replicated
    bf_obj = din("bfus_obj_rep", [128, H], F32)

    out_obj = nc.dram_tensor("out_obj", [NSH, H], F32, kind="ExternalOutput")
    out_rel = nc.dram_tensor("out_rel", [EC, H], F32, kind="ExternalOutput")

    with tile.TileContext(nc) as tc:
        with (
            tc.tile_pool(name="const", bufs=1) as const,
            tc.tile_pool(name="relbuf", bufs=ET) as relbuf,
            tc.tile_pool(name="ownbuf", bufs=2) as ownbuf,
            tc.tile_pool(name="stream", bufs=5) as stream,
            tc.tile_pool(name="wo", bufs=3) as wo,
            tc.tile_pool(name="eb", bufs=7) as eb,
            tc.tile_pool(name="ft", bufs=6) as ft,
            tc.tile_pool(name="small", bufs=4) as small,
            tc.tile_pool(name="dram", bufs=2, space="DRAM") as dram,
        ):
            ident = const.tile([128, 128], BF)
            make_identity(nc, ident)

            # resident weights / constants
            w1_sb = const.tile([128, 2 * KH, 2 * F], BF)
            nc.sync.dma_start(w1_sb, w_pair1.rearrange("(o p) m -> p o m", p=128))
            w2_sb = const.tile([128, 2 * KH, 2 * F], BF)
            nc.sync.dma_start(w2_sb, w_pair2.rearrange("(o p) m -> p o m", p=128))
            bz1_sb = const.tile([128, 1], F32)
            nc.sync.dma_start(bz1_sb, bz1[:])
            bz2_sb = const.tile([128, 1], F32)
            nc.sync.dma_start(bz2_sb, bz2[:])
            onesb_sb = const.tile([128, 2], F32)
            nc.sync.dma_start(onesb_sb, ones_blk[:])
            brd_sb = const.tile([128, H], BF)
            nc.sync.dma_start(brd_sb, brd[:])
            bod_sb = const.tile([128, H], BF)
            nc.sync.dma_start(bod_sb, bod[:])
            bfr_sb = const.tile([128, H], F32)
            nc.sync.dma_start(bfr_sb, bf_rel[:])
            bfo_sb = const.tile([128, H], F32)
            nc.sync.dma_start(bfo_sb, bf_obj[:])
            sidx_sb = const.tile([128, ET], I32)
            nc.sync.dma_start(sidx_sb, sidx.rearrange("(o p) x -> p (o x)", p=128))
            oidx_sb = const.tile([128, ET], I32)
            nc.sync.dma_start(oidx_sb, oidx.rearrange("(o p) x -> p (o x)", p=128))
            sidx2_sb = const.tile([128, ET], I32)
            nc.sync.dma_start(sidx2_sb,
                              sidx2.rearrange("(o p) x -> p (o x)", p=128))
            oidx2_sb = const.tile([128, ET], I32)
            nc.sync.dma_start(oidx2_sb,
                              oidx2.rearrange("(o p) x -> p (o x)", p=128))
            zrow = const.tile([128, ACW], BF)
            nc.vector.memset(zrow, 0.0)

            # ---------------- down projections ----------------
            rel_tiles = [relbuf.tile([128, H], BF, tag="relt", name=f"relt{i}")
                         for i in range(ET)]

            def down_proj(psD, feat_t, wdown, bias_rep, ntiles, out_tiles):
                # groups of 4 output tiles; stream weight k-chunks (split
                # each weight row-block into two half-H DMAs for queue
                # parallelism)
                for g in range(0, ntiles, 4):
                    gw = min(4, ntiles - g)
                    pts = [psD.tile([128, H], F32, tag="dp", name=f"dp{i}")
                           for i in range(gw)]
                    for k in range(KP):
                        wt = stream.tile([128, H], BF, tag="wdown")
                        for hh in range(2):
                            nc.sync.dma_start(
                                wt[:, hh * 512:(hh + 1) * 512],
                                wdown[k * 128:(k + 1) * 128,
                                      hh * 512:(hh + 1) * 512])
                        xt = stream.tile([128, 128 * gw], BF, tag="xdown")
                        nc.sync.dma_start(
                            xt, feat_t[k * 128:(k + 1) * 128,
                                       g * 128:g * 128 + 128 * gw])
                        for i in range(gw):
                            for hh in range(2):
                                nc.tensor.matmul(
                                    out=pts[i][:, hh * 512:(hh + 1) * 512],
                                    lhsT=xt[:, i * 128:(i + 1) * 128],
                                    rhs=wt[:, hh * 512:(hh + 1) * 512],
                                    start=(k == 0), stop=(k == KP - 1))
                    for i in range(gw):
                        ot = out_tiles[g + i]
                        nc.vector.tensor_tensor(
                            out=ot, in0=pts[i], in1=bias_rep,
                            op=mybir.AluOpType.add)
                        nc.vector.tensor_scalar_max(ot, ot, 0.0)

            obj_nm = [eb.tile([128, H], BF, tag="eb", name=f"objnm{i}")
                      for i in range(NT)]
            with tc.tile_pool(name="psD", bufs=4, space="PSUM") as psD:
                down_proj(psD, relft, wrd, brd_sb, ET, rel_tiles)
                down_proj(psD, objft, wod, bod_sb, NT, obj_nm)

            # fusion weights load after the down-proj weight pool is freed
            wih_r_sb = const.tile([128, KH, H], BF)
            nc.sync.dma_start(wih_r_sb,
                              wih_rel.rearrange("(o p) m -> p o m", p=128))
            whh_r_sb = const.tile([128, KH, H], BF)
            nc.sync.dma_start(whh_r_sb,
                              whh_rel.rearrange("(o p) m -> p o m", p=128))

            from contextlib import ExitStack
            _ps_stack = ExitStack()
            psA = _ps_stack.enter_context(
                tc.tile_pool(name="psA", bufs=4, space="PSUM"))
            psT = _ps_stack.enter_context(
                tc.tile_pool(name="psT", bufs=4, space="PSUM"))

            # own_t: relu'd hidden, feature-major [128, KH, NSH]
            own_t = ownbuf.tile([128, KH, NSH], BF, tag="own")
            ag_in = dram.tile([NSH, H], BF)
            for ntl in range(NT):
                nc.sync.dma_start(ag_in[ntl * 128:(ntl + 1) * 128, :], obj_nm[ntl])
                for b in range(2):
                    tpb = psT.tile([128, 4, 128], BF, tag="tp")
                    for j in range(4):
                        c = b * 4 + j
                        nc.tensor.transpose(
                            tpb[:, j, :],
                            obj_nm[ntl][:, c * 128:(c + 1) * 128], ident)
                    nc.scalar.activation(
                        own_t[:, b * 4:(b + 1) * 4,
                              ntl * 128:(ntl + 1) * 128], tpb,
                        mybir.ActivationFunctionType.Relu)

            table = dram.tile([N, H], BF, addr_space="Shared")
            nc.gpsimd.collective_compute(
                "AllGather", mybir.AluOpType.bypass, replica_groups=RG,
                ins=[ag_in.opt()], outs=[table.opt()])

            # ---------------- iterations ----------------
            for it in range(NITER):
                last = it == NITER - 1
                acc = dram.tile([2 * N, ACW], BF, tag="acc")
                for r in range(2 * N // 128):
                    nc.sync.dma_start(acc[r * 128:(r + 1) * 128, :], zrow)

                for et in range(ET):
                    relt = rel_tiles[et]
                    subh = eb.tile([128, H], BF, tag="eb", name="subh")
                    nc.gpsimd.indirect_dma_start(
                        out=subh, out_offset=None, in_=table[:, :],
                        in_offset=bass.IndirectOffsetOnAxis(
                            ap=sidx_sb[:, et:et + 1], axis=0))
                    objh = eb.tile([128, H], BF, tag="eb", name="objh")
                    nc.gpsimd.indirect_dma_start(
                        out=objh, out_offset=None, in_=table[:, :],
                        in_offset=bass.IndirectOffsetOnAxis(
                            ap=oidx_sb[:, et:et + 1], axis=0))

                    # layernorm stats + gates for the two MPU pairs
                    gts = []
                    for pair_i, (oth, w_sb, bz_sb) in enumerate(
                            [(subh, w1_sb, bz1_sb), (objh, w2_sb, bz2_sb)]):
                        st = small.tile([128, 4, 6], F32, tag="bnst")
                        r3 = relt.rearrange("p (a b) -> p a b", b=512)
                        o3 = oth.rearrange("p (a b) -> p a b", b=512)
                        nc.vector.bn_stats(st[:, 0, :], r3[:, 0, :])
                        nc.vector.bn_stats(st[:, 1, :], r3[:, 1, :])
                        nc.vector.bn_stats(st[:, 2, :], o3[:, 0, :])
                        nc.vector.bn_stats(st[:, 3, :], o3[:, 1, :])
                        mv = small.tile([128, 2], F32, tag="bnmv")
                        nc.vector.bn_aggr(mv, st)
                        sd = small.tile([128, 1], F32, tag="sd")
                        nc.vector.tensor_scalar_add(sd, mv[:, 1:2], EPS)
                        nc.scalar.sqrt(sd, sd)
                        rstd = small.tile([128, 1], F32, tag="rstd")
                        nc.vector.reciprocal(rstd, sd)

                        # normalized tiles (edge-major)
                        xa = eb.tile([128, H], BF, tag="eb", name="xa")
                        nc.vector.tensor_scalar(
                            out=xa, in0=relt, scalar1=mv[:, 0:1], scalar2=rstd,
                            op0=mybir.AluOpType.subtract, op1=mybir.AluOpType.mult)
                        xb = eb.tile([128, H], BF, tag="eb", name="xb")
                        nc.vector.tensor_scalar(
                            out=xb, in0=oth, scalar1=mv[:, 0:1], scalar2=rstd,
                            op0=mybir.AluOpType.subtract, op1=mybir.AluOpType.mult)

                        # gate logits: transpose+relu in 4-chunk batches
                        zp = psT.tile([128, 128], F32, tag="tp")
                        for b in range(4):
                            tpb = psT.tile([128, 4, 128], BF, tag="tp")
                            for j in range(4):
                                c = b * 4 + j
                                src = xa if c < KH else xb
                                cc = c if c < KH else c - KH
                                nc.tensor.transpose(
                                    tpb[:, j, :],
                                    src[:, cc * 128:(cc + 1) * 128], ident)
                            xch = ft.tile([128, 4, 128], BF, tag="ft",
                                          name="xch")
                            if b % 2 == pair_i % 2:
                                nc.scalar.activation(
                                    xch, tpb,
                                    mybir.ActivationFunctionType.Relu)
                            else:
                                nc.vector.tensor_scalar_max(xch, tpb, 0.0)
                            for j in range(4):
                                c = b * 4 + j
                                nc.tensor.matmul(
                                    out=zp, lhsT=w_sb[:, c, :],
                                    rhs=xch[:, j, :],
                                    start=(c == 0), stop=(c == 2 * KH - 1))
                        zs = small.tile([128, 128], F32, tag="zs")
                        nc.scalar.activation(
                            zs, zp, mybir.ActivationFunctionType.Sigmoid,
                            bias=bz_sb)
                        gt_ps = psT.tile([128, 2], F32, tag="tp")
                        nc.tensor.matmul(out=gt_ps, lhsT=zs, rhs=onesb_sb,
                                         start=True, stop=True)
                        gt = small.tile([128, 2], F32, tag="gt")
                        nc.vector.tensor_copy(gt, gt_ps)
                        gts.append(gt)

                    # scatter messages (gates pre-scaled by 1/128 = mean*0.5)
                    for gt, idx_sb in ((gts[0], sidx2_sb),
                                       (gts[1], oidx2_sb)):
                        m = eb.tile([128, ACW], BF, tag="eb", name="mscat")
                        nc.vector.tensor_scalar_mul(m[:, 0:H], relt, gt[:, 1:2])
                        nc.vector.memset(m[:, H:H + 1], 1.0)
                        nc.vector.memset(m[:, H + 1:ACW], 0.0)
                        nc.gpsimd.indirect_dma_start(
                            out=acc[:, :], out_offset=bass.IndirectOffsetOnAxis(
                                ap=idx_sb[:, et:et + 1], axis=0),
                            in_=m, in_offset=None,
                            compute_op=mybir.AluOpType.add)

                    # rel message: subh*g_s2p/2 + objh*g_o2p/2
                    t1 = eb.tile([128, H], BF, tag="eb", name="t1")
                    nc.vector.tensor_scalar_mul(t1, subh, gts[0][:, 0:1])
                    msg = eb.tile([128, H], BF, tag="eb", name="msg")
                    nc.vector.tensor_scalar_mul(msg, objh, gts[1][:, 0:1])
                    nc.vector.tensor_tensor(out=msg, in0=msg, in1=t1,
                                            op=mybir.AluOpType.add)

                    # rel fusion: per chunk transpose+relu msg / relt, matmul
                    fph = [psA.tile([128, 512], F32, tag="fus", name=f"fph{i}")
                           for i in range(2)]
                    for b in range(2):
                        tpm = psT.tile([128, 4, 128], BF, tag="tp")
                        tph = psT.tile([128, 4, 128], BF, tag="tp")
                        for j in range(4):
                            c = b * 4 + j
                            nc.tensor.transpose(
                                tpm[:, j, :], msg[:, c * 128:(c + 1) * 128],
                                ident)
                            nc.tensor.transpose(
                                tph[:, j, :], relt[:, c * 128:(c + 1) * 128],
                                ident)
                        mch = ft.tile([128, 4, 128], BF, tag="ft", name="mch")
                        nc.scalar.activation(
                            mch, tpm, mybir.ActivationFunctionType.Relu)
                        hch = ft.tile([128, 4, 128], BF, tag="ft", name="hch")
                        nc.vector.tensor_scalar_max(hch, tph, 0.0)
                        for j in range(4):
                            c = b * 4 + j
                            for hh in range(2):
                                sl = slice(hh * 512, (hh + 1) * 512)
                                nc.tensor.matmul(
                                    out=fph[hh], lhsT=mch[:, j, :],
                                    rhs=wih_r_sb[:, c, sl],
                                    start=(c == 0), stop=False)
                                nc.tensor.matmul(
                                    out=fph[hh], lhsT=hch[:, j, :],
                                    rhs=whh_r_sb[:, c, sl],
                                    start=False, stop=(c == KH - 1))
                    if last:
                        fo = eb.tile([128, H], F32, tag="f32w", name="fo")
                        for hh in range(2):
                            sl = slice(hh * 512, (hh + 1) * 512)
                            nc.vector.tensor_tensor(
                                out=fo[:, sl], in0=fph[hh], in1=bfr_sb[:, sl],
                                op=mybir.AluOpType.add)
                        nc.sync.dma_start(out_rel[et * 128:(et + 1) * 128, :], fo)
                    else:
                        for hh in range(2):
                            sl = slice(hh * 512, (hh + 1) * 512)
                            nc.vector.tensor_tensor(
                                out=relt[:, sl], in0=fph[hh], in1=bfr_sb[:, sl],
                                op=mybir.AluOpType.add)

                # ---- node update ----
                rs_a = dram.tile([2 * NSH, ACW], BF, tag="rsa")
                nc.gpsimd.collective_compute(
                    "ReduceScatter", mybir.AluOpType.add, replica_groups=RG,
                    ins=[acc.opt()], outs=[rs_a.opt()])
                rs_3 = rs_a.rearrange("(n t) w -> n t w", t=2)

                if not last:
                    ag_in2 = dram.tile([NSH, H], BF, tag="agin")
                    new_own = ownbuf.tile([128, KH, NSH], BF, tag="own")
                for ntl in range(NT):
                    asb = eb.tile([128, ACW], BF, tag="eb", name="asb")
                    nc.sync.dma_start(asb, rs_3[ntl * 128:(ntl + 1) * 128, 0, :])
                    aob = eb.tile([128, ACW], BF, tag="eb", name="aob")
                    nc.sync.dma_start(aob, rs_3[ntl * 128:(ntl + 1) * 128, 1, :])
                    msgn = eb.tile([128, H], F32, tag="f32w", name="msgn")
                    tmpn = eb.tile([128, H], F32, tag="f32w", name="tmpn")
                    for src, dst in ((asb, msgn), (aob, tmpn)):
                        cnt = small.tile([128, 1], F32, tag="cnt")
                        nc.vector.tensor_copy(cnt, src[:, H:H + 1])
                        nc.vector.tensor_scalar_max(cnt, cnt, 1.0)
                        rc = small.tile([128, 1], F32, tag="rc")
                        nc.vector.reciprocal(rc, cnt)
                        nc.vector.tensor_scalar_mul(dst, src[:, 0:H], rc)
                    nc.vector.tensor_tensor(out=msgn, in0=msgn, in1=tmpn,
                                            op=mybir.AluOpType.add)
                    msgr = eb.tile([128, H], BF, tag="eb", name="msgr")
                    nc.vector.tensor_scalar_max(msgr, msgn, 0.0)
                    fph = [psA.tile([128, 512], F32, tag="fus", name=f"fphn{i}")
                           for i in range(2)]
                    mchs = []
                    for b in range(2):
                        tpb = psT.tile([128, 4, 128], BF, tag="tp")
                        for j in range(4):
                            c = b * 4 + j
                            nc.tensor.transpose(
                                tpb[:, j, :], msgr[:, c * 128:(c + 1) * 128],
                                ident)
                        mchb = ft.tile([128, 4, 128], BF, tag="ft", name="mchn")
                        nc.vector.tensor_copy(mchb, tpb)
                        mchs.append(mchb)
                    for c in range(KH):
                        mch = mchs[c // 4][:, c % 4, :]
                        for hh in range(2):
                            sl = slice(hh * 512, (hh + 1) * 512)
                            wi = wo.tile([128, 512], BF, tag="wo", name="wi")
                            nc.sync.dma_start(
                                wi, wih_obj[c * 128:(c + 1) * 128, sl])
                            wh = wo.tile([128, 512], BF, tag="wo", name="wh")
                            nc.sync.dma_start(
                                wh, whh_obj[c * 128:(c + 1) * 128, sl])
                            nc.tensor.matmul(
                                out=fph[hh], lhsT=mch, rhs=wi,
                                start=(c == 0), stop=False)
                            nc.tensor.matmul(
                                out=fph[hh],
                                lhsT=own_t[:, c, ntl * 128:(ntl + 1) * 128],
                                rhs=wh,
                                start=False, stop=(c == KH - 1))
                    if last:
                        onew = eb.tile([128, H], F32, tag="f32w", name="onew")
                        for hh in range(2):
                            sl = slice(hh * 512, (hh + 1) * 512)
                            nc.vector.tensor_tensor(
                                out=onew[:, sl], in0=fph[hh], in1=bfo_sb[:, sl],
                                op=mybir.AluOpType.add)
                        nc.sync.dma_start(out_obj[ntl * 128:(ntl + 1) * 128, :],
                                          onew)
                    else:
                        onb = eb.tile([128, H], BF, tag="eb", name="onb")
                        for hh in range(2):
                            sl = slice(hh * 512, (hh + 1) * 512)
                            nc.vector.tensor_tensor(
                                out=onb[:, sl], in0=fph[hh], in1=bfo_sb[:, sl],
                                op=mybir.AluOpType.add)
                        nc.sync.dma_start(
                            ag_in2[ntl * 128:(ntl + 1) * 128, :], onb)
                        for b in range(2):
                            tpb = psT.tile([128, 4, 128], BF, tag="tp")
                            for j in range(4):
                                c = b * 4 + j
                                nc.tensor.transpose(
                                    tpb[:, j, :],
                                    onb[:, c * 128:(c + 1) * 128], ident)
                            nc.scalar.activation(
                                new_own[:, b * 4:(b + 1) * 4,
                                        ntl * 128:(ntl + 1) * 128], tpb,
                                mybir.ActivationFunctionType.Relu)
                if not last:
                    table2 = dram.tile([N, H], BF, tag="table2", addr_space="Shared")
                    nc.gpsimd.collective_compute(
                        "AllGather", mybir.AluOpType.bypass, replica_groups=RG,
                        ins=[ag_in2.opt()], outs=[table2.opt()])
                    table = table2
                    own_t = new_own
            _ps_stack.close()
    return nc


def _prep_inputs(inputs):
    f = {k: np.asarray(v) for k, v in inputs.items()}
    relT = np.ascontiguousarray(f["rel_feat"].astype(BF_NP).T)      # [PDIM, E]
    objT = np.ascontiguousarray(f["obj_feat"].astype(BF_NP).T)      # [PDIM, N]

    def reord(w):
        return np.concatenate([w[H:], w[:H]], axis=0)

    w1 = np.concatenate([f["w_s2p"], reord(f["w_p2s"])], axis=1).astype(BF_NP)
    w2 = np.concatenate([f["w_o2p"], reord(f["w_p2o"])], axis=1).astype(BF_NP)
    bz1 = np.concatenate([f["b_s2p"], f["b_p2s"]]).astype(np.float32)[:, None]
    bz2 = np.concatenate([f["b_o2p"], f["b_p2o"]]).astype(np.float32)[:, None]
    ones_blk = np.zeros((2 * F, 2), np.float32)
    ones_blk[:F, 0] = 1.0 / 128.0
    ones_blk[F:, 1] = 1.0 / 128.0

    def rep(b, dt=np.float32):
        return np.tile(np.asarray(b).astype(dt)[None, :], (128, 1))

    common = {
        "w_rel_down": f["w_rel_down"].astype(BF_NP),
        "w_obj_down": f["w_obj_down"].astype(BF_NP),
        "b_rel_down_rep": rep(f["b_rel_down"], BF_NP),
        "b_obj_down_rep": rep(f["b_obj_down"], BF_NP),
        "w_pair1": w1, "w_pair2": w2,
        "bz_pair1": bz1, "bz_pair2": bz2, "ones_blk": ones_blk,
        "wih_relf": f["wih_relf"].astype(BF_NP),
        "whh_relf": f["whh_relf"].astype(BF_NP),
        "wih_objf": f["wih_objf"].astype(BF_NP),
        "whh_objf": f["whh_objf"].astype(BF_NP),
        "bfus_rel_rep": rep(f["bih_relf"] + f["bhh_relf"]),
        "bfus_obj_rep": rep(f["bih_objf"] + f["bhh_objf"]),
    }
    maps = []
    for c in range(NCORES):
        m = dict(common)
        m["rel_feat_t"] = np.ascontiguousarray(relT[:, c * EC:(c + 1) * EC])
        m["obj_feat_t"] = np.ascontiguousarray(objT[:, c * NSH:(c + 1) * NSH])
        si = f["sub_idx"][c * EC:(c + 1) * EC].astype(np.int32)[:, None]
        oi = f["obj_idx"][c * EC:(c + 1) * EC].astype(np.int32)[:, None]
        m["sub_idx"] = np.ascontiguousarray(si)
        m["obj_idx"] = np.ascontiguousarray(oi)
        m["sub_idx2"] = np.ascontiguousarray(2 * si)
        m["obj_idx2"] = np.ascontiguousarray(2 * oi + 1)
        maps.append(m)
    return maps


def _run(inputs, trace=False):
    nc = bacc.Bacc(None, target_bir_lowering=False)
    _build(nc)
    nc.compile()
    maps = _prep_inputs(inputs)
    res = run_bass_kernel_spmd(nc, maps, core_ids=list(range(NCORES)),
                               trace=trace)
    outs = res.results
    obj = np.concatenate([np.asarray(outs[c]["out_obj"], np.float32)
                          for c in range(NCORES)], axis=0)
    rel = np.concatenate([np.asarray(outs[c]["out_rel"], np.float32)
                          for c in range(NCORES)], axis=0)
    full = np.concatenate([obj, rel], axis=0)
    return full, res


def kernel(**inputs):
    full, _ = _run(inputs, trace=False)
    return full
